# revision 41
# baseline (speedup 1.0000x reference)
"""Causal single-head attention (B=4, T=4096, C=128) on 8 Trainium2 cores.

Sharding: core c -> (batch b = c//2, parity h = c%2). Each core owns the
query row-blocks {128*(2i+h) : i=0..15} of its batch — an interleaved split
that balances causal work exactly and keeps the SPMD program identical
across cores (all per-core differences are input data, incl. the causal
mask for the last two key blocks of each row-block).

Per query block tb = 2i+h the core processes keys s < 128*(2i+2) (for h=0
the final key block is fully masked — wasted but uniform). Pipeline per
block i: QK^T matmuls (bf16, N=512 chunks into PSUM) -> causal mask added
by a tiny identity @ mask matmul accumulating into the same PSUM region ->
exp on ACT (PSUM -> SBUF bf16) with accumulated row-sum partials -> DMA-
xbar transpose of P into a 4-block group buffer. Per group of 4 blocks:
fat P^T @ V matmuls (V stationary, N up to 512) accumulate O^T[c, t] in
PSUM, which is copied out and shipped UNNORMALIZED and UNTRANSPOSED; the
host divides by the row sums and transposes. PV work for a group is
drained between later iterations so the in-order engine streams overlap.
"""

import sys

sys.path.insert(0, "/opt/trn_rl_repo")

import numpy as np
import ml_dtypes
from contextlib import ExitStack

import concourse.bass as bass
import concourse.mybir as mybir
import concourse.tile as tile
from concourse import bacc
from concourse.bass_utils import run_bass_kernel_spmd

B, T, C = 4, 4096, 128
N_CORES = 8
NTB = 16  # own 128-row query blocks per core
BF16 = mybir.dt.bfloat16
F32 = mybir.dt.float32
SCALE = float(1.0 / np.sqrt(C))
NEG = -1.0e30
ST_COLS = 1024  # PSUM score-tile width (2 banks)

_cached = {}


def s_len_of(i: int) -> int:
    return 128 * (2 * i + 2)


def ngr_of(i: int) -> int:
    return (s_len_of(i) + ST_COLS - 1) // ST_COLS


def build():
    nc = bacc.Bacc("TRN2", target_bir_lowering=False, debug=False)

    xt = nc.dram_tensor("xt", [C, T], BF16, kind="ExternalInput").ap()
    xq = nc.dram_tensor("xq", [C, NTB * 128], BF16, kind="ExternalInput").ap()
    cpack = nc.dram_tensor("cpack", [128, 640], BF16, kind="ExternalInput").ap()
    # outputs: O^T per group (host transposes + normalizes) and rowsum partials
    y = nc.dram_tensor("y", [C, NTB * 128], F32, kind="ExternalOutput").ap()
    rsum = nc.dram_tensor("rsum", [128, NTB * 4], F32, kind="ExternalOutput").ap()

    with ExitStack() as ctx:
        tc = ctx.enter_context(tile.TileContext(nc))
        consts = ctx.enter_context(tc.tile_pool(name="consts", bufs=1))
        pnat_pool = ctx.enter_context(tc.tile_pool(name="pnat", bufs=5))
        pt_pool = ctx.enter_context(tc.tile_pool(name="pt", bufs=1))
        osb_pool = ctx.enter_context(tc.tile_pool(name="osb", bufs=2))
        st_pool = ctx.enter_context(tc.tile_pool(name="st", bufs=3, space="PSUM"))
        ot_pool = ctx.enter_context(tc.tile_pool(name="ot", bufs=2, space="PSUM"))

        # ---- persistent SBUF tensors ----
        xt_sb = consts.tile([C, T], BF16, tag="xt_sb")
        xq_sb = consts.tile([C, NTB * 128], BF16, tag="xq_sb")
        cp_sb = consts.tile([128, 640], BF16, tag="cp_sb")
        m2_sb = cp_sb[:, 0:256]
        id_sb = cp_sb[:, 256:384]
        mqk_sb = cp_sb[:, 384:512]
        wv_sb = cp_sb[:, 512:640]
        qT_sb = consts.tile([C, NTB * 128], BF16, tag="qT_sb")  # [c, t_local]
        v_sb = consts.tile([C, T], BF16, tag="v_sb")  # [s%128, s_blk*128 + c]
        racc_sb = consts.tile([128, NTB, 4], F32, tag="racc_sb")  # rowsum partials

        nc.sync.dma_start(out=cp_sb, in_=cpack)
        nc.sync.dma_start(out=xq_sb, in_=xq)
        for g in range(2):
            nc.sync.dma_start(
                out=xt_sb[:, g * 2048 : (g + 1) * 2048],
                in_=xt[:, g * 2048 : (g + 1) * 2048],
            )

        # ---- projection emitters (interleaved into the main loop) ----
        # q'^T[c, t] = sum_i M[i, c] * XQ[i, t]   (M = Wq^T Wk folded on host;
        # the key side is then raw X^T, so there is no K projection at all)
        def q_proj(g):
            st = st_pool.tile([128, ST_COLS], F32, tag="st", name="st")
            nc.tensor.matmul(
                st[:, :512],
                lhsT=mqk_sb,
                rhs=xq_sb[:, g * 512 : (g + 1) * 512],
                start=True,
                stop=True,
            )
            nc.vector.tensor_copy(qT_sb[:, g * 512 : (g + 1) * 512], st[:, :512])

        # V natural: V[s0+p, c] = sum_cin XT[cin, s0+p] * WvT[cin, c]
        def v_proj(g):
            st = st_pool.tile([128, ST_COLS], F32, tag="st", name="st")
            for m in range(4):
                j = 4 * g + m
                nc.tensor.matmul(
                    st[:, m * 128 : (m + 1) * 128],
                    lhsT=xt_sb[:, j * 128 : (j + 1) * 128],
                    rhs=wv_sb,
                    start=True,
                    stop=True,
                )
            nc.vector.tensor_copy(v_sb[:, g * 512 : (g + 1) * 512], st[:, :512])

        # ---- main loop pieces ----
        pts = {}  # group -> pt tile
        tp_pending = []  # deferred PE-transpose jobs for small groups

        def qk_exp_xbar(i):
            s_len = s_len_of(i)
            nblk = s_len // 128
            ngr = ngr_of(i)
            g, il = i // 4, i % 4

            p_nat = pnat_pool.tile([128, T], BF16, tag="p_nat")

            for gg in range(ngr):
                g0 = gg * ST_COLS
                glen = min(ST_COLS, s_len - g0)
                st = st_pool.tile([128, ST_COLS], F32, tag="st")
                off = 0
                while off < glen:
                    w = min(512, glen - off)
                    last = gg == ngr - 1 and off + w == glen
                    nc.tensor.matmul(
                        st[:, off : off + w],
                        lhsT=qT_sb[:, i * 128 : (i + 1) * 128],
                        rhs=xt_sb[:, g0 + off : g0 + off + w],
                        start=True,
                        stop=not last,
                        skip_group_check=True,
                    )
                    off += w
                if gg == ngr - 1:
                    # causal mask: st[:, glen-256:glen] += I^T @ m2
                    nc.tensor.matmul(
                        st[:, glen - 256 : glen],
                        lhsT=id_sb,
                        rhs=m2_sb,
                        start=False,
                        stop=True,
                        skip_group_check=True,
                    )
                nc.scalar.activation(
                    out=p_nat[:, g0 : g0 + glen],
                    in_=st[:, :glen],
                    func=mybir.ActivationFunctionType.Exp,
                    scale=SCALE,
                    accum_out=racc_sb[:, i, gg : gg + 1],
                )

            if g not in pts:
                pts[g] = pt_pool.tile(
                    [128, 8 * g + 8, 512], BF16, tag=f"pt{g}", name="pt"
                )
            # transpose P [t128, s_len] -> [s%128, s_blk, t128] into group buf.
            # Small groups go via PE transposes + DVE copies (deferred one
            # iteration) to take load off the congested SP xbar queue.
            if i <= 7:
                tp_pending.append((g, il, nblk, p_nat))
            else:
                nc.sync.dma_start_transpose(
                    out=pts[g][:, :nblk, il * 128 : (il + 1) * 128],
                    in_=p_nat[:, :s_len],
                )

        def tp_drain():
            while tp_pending:
                g, il, nblk, p_nat = tp_pending.pop(0)
                for m in range((nblk + 3) // 4):
                    cnt = min(4, nblk - 4 * m)
                    stx = st_pool.tile([128, ST_COLS], F32, tag="st", name="stx")
                    sbv = stx.bitcast(BF16)
                    for k in range(cnt):
                        j = 4 * m + k
                        nc.tensor.transpose(
                            sbv[:, k * 128 : (k + 1) * 128],
                            p_nat[:, j * 128 : (j + 1) * 128],
                            id_sb,
                        )
                    nc.vector.tensor_copy(
                        pts[g][:, 4 * m : 4 * m + cnt, il * 128 : (il + 1) * 128],
                        sbv[:, : cnt * 128].rearrange("p (a b) -> p a b", b=128),
                    )

        ots = {}

        def pv_job(g, j):
            """One P^T@V matmul for group g, key-block j."""
            nblk = 8 * g + 8
            if j == 0:
                ots[g] = ot_pool.tile([128, 512], F32, tag="ot", name="ot")
            off = 0 if j < 8 * g + 2 else 128 * ((j - (8 * g + 2)) // 2 + 1)
            nc.tensor.matmul(
                ots[g][:, off:512],
                lhsT=v_sb[:, j * 128 : (j + 1) * 128],
                rhs=pts[g][:, j, off:512],
                start=(j == 0),
                stop=(j == nblk - 1),
            )

        def epilogue(g):
            otsb = osb_pool.tile([128, 512], F32, tag="otsb", name="otsb")
            nc.vector.tensor_copy(otsb, ots[g])
            nc.sync.dma_start(out=y[:, g * 512 : (g + 1) * 512], in_=otsb)

        # ---- emission schedule: fast-start order, PV drains between iters ----
        order = [3] + list(range(NTB - 1, 3, -1)) + [2, 1, 0]
        pv_queue = []
        epi_pending = []
        state = {"done": set(), "ready": []}

        nv = {"v": 0}

        def drain(n):
            for _ in range(n):
                if not pv_queue or pv_queue[0][1] >= 4 * nv["v"]:
                    break
                g, j = pv_queue.pop(0)
                pv_job(g, j)
                if j == 8 * g + 7:
                    epi_pending.append(g)

        q_proj(0)  # qT cols for i=3 live in chunk 0
        for g in range(1, 4):
            q_proj(g)

        for idx, i in enumerate(order):
            for g in state["ready"]:
                pv_queue.extend((g, j) for j in range(8 * g + 8))
            state["ready"] = []
            if nv["v"] < 8:
                v_proj(nv["v"])
                nv["v"] += 1
            if epi_pending:
                epilogue(epi_pending.pop(0))
            tp_drain()
            qk_exp_xbar(i)
            state["done"].add(i)
            g = i // 4
            if all(4 * g + m in state["done"] for m in range(4)):
                state["ready"].append(g)
            drain(13)
        for g in state["ready"]:
            pv_queue.extend((g, j) for j in range(8 * g + 8))
        nv["v"] = 8
        tp_drain()
        while pv_queue:
            drain(10)
            if epi_pending:
                epilogue(epi_pending.pop(0))
        while epi_pending:
            epilogue(epi_pending.pop(0))
        # ship rowsum partials
        nc.sync.dma_start(out=rsum, in_=racc_sb.rearrange("p i g -> p (i g)"))

    nc.compile()
    return nc


def make_mask2(h: int) -> np.ndarray:
    m = np.zeros((128, 256), np.float32)
    ii = np.arange(128)[:, None]
    jj = np.arange(128)[None, :]
    tri = np.where(jj <= ii, 0.0, NEG).astype(np.float32)
    if h == 0:
        m[:, 0:128] = tri
        m[:, 128:256] = NEG
    else:
        m[:, 0:128] = 0.0
        m[:, 128:256] = tri
    return m.astype(ml_dtypes.bfloat16)


def own_rows(h: int) -> np.ndarray:
    blocks = 2 * np.arange(NTB) + h
    return (blocks[:, None] * 128 + np.arange(128)[None, :]).reshape(-1)


def prep_inputs(X, Wq, Wk, Wv):
    bf = ml_dtypes.bfloat16
    in_maps = []
    for c in range(N_CORES):
        b, h = c // 2, c % 2
        rows = own_rows(h)
        xb = np.asarray(X[b], np.float32)
        in_maps.append(
            {
                "xt": np.ascontiguousarray(xb.T).astype(bf),
                "xq": np.ascontiguousarray(xb[rows].T).astype(bf),
                "cpack": np.concatenate(
                    [
                        make_mask2(h),
                        np.eye(128, dtype=np.float32).astype(bf),
                        (np.asarray(Wq, np.float32).T @ np.asarray(Wk, np.float32)).astype(bf),
                        np.ascontiguousarray(np.asarray(Wv, np.float32).T).astype(bf),
                    ],
                    axis=1,
                ),
            }
        )
    return in_maps


def kernel(X, Wq, Wk, Wv, _trace=False, _tmpdir=None):
    if "nc" not in _cached:
        _cached["nc"] = build()
    nc = _cached["nc"]
    in_maps = prep_inputs(X, Wq, Wk, Wv)
    res = run_bass_kernel_spmd(
        nc,
        in_maps,
        list(range(N_CORES)),
        trace=_trace,
        **({"tmpdir": _tmpdir} if _tmpdir else {}),
    )
    out = np.empty((B, T, C), np.float32)
    for c in range(N_CORES):
        b, h = c // 2, c % 2
        ot = res.results[c]["y"]  # [128, 2048] = O^T unnormalized
        rp = res.results[c]["rsum"].reshape(128, NTB, 4)  # partial rowsums
        r = np.zeros((128, NTB), np.float32)
        for i in range(NTB):
            r[:, i] = rp[:, i, : ngr_of(i)].sum(axis=1)
        o = np.ascontiguousarray(ot.T).reshape(NTB, 128, C) / r.T[:, :, None]
        out[b, own_rows(h)] = o.reshape(NTB * 128, C)
    _cached["last_res"] = res
    return out


# revision 42
# speedup vs baseline: 1.0185x; 1.0185x over previous
"""Causal single-head attention (B=4, T=4096, C=128) on 8 Trainium2 cores.

Sharding: core c -> (batch b = c//2, parity h = c%2). Each core owns the
query row-blocks {128*(2i+h) : i=0..15} of its batch — an interleaved split
that balances causal work exactly and keeps the SPMD program identical
across cores (all per-core differences are input data, incl. the causal
mask for the last two key blocks of each row-block).

Per query block tb = 2i+h the core processes keys s < 128*(2i+2) (for h=0
the final key block is fully masked — wasted but uniform). Pipeline per
block i: QK^T matmuls (bf16, N=512 chunks into PSUM) -> causal mask added
by a tiny identity @ mask matmul accumulating into the same PSUM region ->
exp on ACT (PSUM -> SBUF bf16) with accumulated row-sum partials -> DMA-
xbar transpose of P into a 4-block group buffer. Per group of 4 blocks:
fat P^T @ V matmuls (V stationary, N up to 512) accumulate O^T[c, t] in
PSUM, which is copied out and shipped UNNORMALIZED and UNTRANSPOSED; the
host divides by the row sums and transposes. PV work for a group is
drained between later iterations so the in-order engine streams overlap.
"""

import sys

sys.path.insert(0, "/opt/trn_rl_repo")

import numpy as np
import ml_dtypes
from contextlib import ExitStack

import concourse.bass as bass
import concourse.mybir as mybir
import concourse.tile as tile
from concourse import bacc
from concourse.bass_utils import run_bass_kernel_spmd

B, T, C = 4, 4096, 128
N_CORES = 8
NTB = 16  # own 128-row query blocks per core
BF16 = mybir.dt.bfloat16
F32 = mybir.dt.float32
SCALE = float(1.0 / np.sqrt(C))
NEG = -1.0e30
ST_COLS = 1024  # PSUM score-tile width (2 banks)

_cached = {}


def s_len_of(i: int) -> int:
    return 128 * (2 * i + 2)


def ngr_of(i: int) -> int:
    return (s_len_of(i) + ST_COLS - 1) // ST_COLS


def build():
    nc = bacc.Bacc("TRN2", target_bir_lowering=False, debug=False)

    xt = nc.dram_tensor("xt", [C, T], BF16, kind="ExternalInput").ap()
    xq = nc.dram_tensor("xq", [C, NTB * 128], BF16, kind="ExternalInput").ap()
    cpack = nc.dram_tensor("cpack", [128, 640], BF16, kind="ExternalInput").ap()
    # outputs: O^T per group (host transposes + normalizes) and rowsum partials
    y = nc.dram_tensor("y", [C, NTB * 128], F32, kind="ExternalOutput").ap()
    rsum = nc.dram_tensor("rsum", [128, NTB * 4], F32, kind="ExternalOutput").ap()

    with ExitStack() as ctx:
        tc = ctx.enter_context(tile.TileContext(nc))
        consts = ctx.enter_context(tc.tile_pool(name="consts", bufs=1))
        pnat_pool = ctx.enter_context(tc.tile_pool(name="pnat", bufs=5))
        pt_pool = ctx.enter_context(tc.tile_pool(name="pt", bufs=1))
        osb_pool = ctx.enter_context(tc.tile_pool(name="osb", bufs=2))
        st_pool = ctx.enter_context(tc.tile_pool(name="st", bufs=3, space="PSUM"))
        ot_pool = ctx.enter_context(tc.tile_pool(name="ot", bufs=2, space="PSUM"))

        # ---- persistent SBUF tensors ----
        xt_sb = consts.tile([C, T], BF16, tag="xt_sb")
        xq_sb = consts.tile([C, NTB * 128], BF16, tag="xq_sb")
        cp_sb = consts.tile([128, 640], BF16, tag="cp_sb")
        m2_sb = cp_sb[:, 0:256]
        id_sb = cp_sb[:, 256:384]
        mqk_sb = cp_sb[:, 384:512]
        wv_sb = cp_sb[:, 512:640]
        qT_sb = consts.tile([C, NTB * 128], BF16, tag="qT_sb")  # [c, t_local]
        v_sb = consts.tile([C, T], BF16, tag="v_sb")  # [s%128, s_blk*128 + c]
        racc_sb = consts.tile([128, NTB, 4], F32, tag="racc_sb")  # rowsum partials

        nc.sync.dma_start(out=cp_sb, in_=cpack)
        nc.sync.dma_start(out=xq_sb, in_=xq)
        for g in range(2):
            nc.sync.dma_start(
                out=xt_sb[:, g * 2048 : (g + 1) * 2048],
                in_=xt[:, g * 2048 : (g + 1) * 2048],
            )

        # ---- projection emitters (interleaved into the main loop) ----
        # q'^T[c, t] = sum_i M[i, c] * XQ[i, t]   (M = Wq^T Wk folded on host;
        # the key side is then raw X^T, so there is no K projection at all)
        def q_proj(g):
            st = st_pool.tile([128, ST_COLS], F32, tag="st", name="st")
            nc.tensor.matmul(
                st[:, :512],
                lhsT=mqk_sb,
                rhs=xq_sb[:, g * 512 : (g + 1) * 512],
                start=True,
                stop=True,
            )
            nc.vector.tensor_copy(qT_sb[:, g * 512 : (g + 1) * 512], st[:, :512])

        # V natural: V[s0+p, c] = sum_cin XT[cin, s0+p] * WvT[cin, c]
        def v_proj(g):
            st = st_pool.tile([128, ST_COLS], F32, tag="st", name="st")
            for m in range(4):
                j = 4 * g + m
                nc.tensor.matmul(
                    st[:, m * 128 : (m + 1) * 128],
                    lhsT=xt_sb[:, j * 128 : (j + 1) * 128],
                    rhs=wv_sb,
                    start=True,
                    stop=True,
                )
            nc.vector.tensor_copy(v_sb[:, g * 512 : (g + 1) * 512], st[:, :512])

        # ---- main loop pieces ----
        pts = {}  # group -> pt tile

        def qk_exp_xbar(i):
            s_len = s_len_of(i)
            nblk = s_len // 128
            ngr = ngr_of(i)
            g, il = i // 4, i % 4

            p_nat = pnat_pool.tile([128, T], BF16, tag="p_nat")

            for gg in range(ngr):
                g0 = gg * ST_COLS
                glen = min(ST_COLS, s_len - g0)
                st = st_pool.tile([128, ST_COLS], F32, tag="st")
                off = 0
                while off < glen:
                    w = min(512, glen - off)
                    last = gg == ngr - 1 and off + w == glen
                    nc.tensor.matmul(
                        st[:, off : off + w],
                        lhsT=qT_sb[:, i * 128 : (i + 1) * 128],
                        rhs=xt_sb[:, g0 + off : g0 + off + w],
                        start=True,
                        stop=not last,
                        skip_group_check=True,
                    )
                    off += w
                if gg == ngr - 1:
                    # causal mask: st[:, glen-256:glen] += I^T @ m2
                    nc.tensor.matmul(
                        st[:, glen - 256 : glen],
                        lhsT=id_sb,
                        rhs=m2_sb,
                        start=False,
                        stop=True,
                        skip_group_check=True,
                    )
                nc.scalar.activation(
                    out=p_nat[:, g0 : g0 + glen],
                    in_=st[:, :glen],
                    func=mybir.ActivationFunctionType.Exp,
                    scale=SCALE,
                    accum_out=racc_sb[:, i, gg : gg + 1],
                )

            if g not in pts:
                pts[g] = pt_pool.tile(
                    [128, 8 * g + 8, 512], BF16, tag=f"pt{g}", name="pt"
                )
            # transpose P [t128, s_len] -> [s%128, s_blk, t128] into group buf
            nc.sync.dma_start_transpose(
                out=pts[g][:, :nblk, il * 128 : (il + 1) * 128],
                in_=p_nat[:, :s_len],
            )

        ots = {}

        def pv_job(g, j):
            """One P^T@V matmul for group g, key-block j."""
            nblk = 8 * g + 8
            if j == 0:
                ots[g] = ot_pool.tile([128, 512], F32, tag="ot", name="ot")
            off = 0 if j < 8 * g + 2 else 128 * ((j - (8 * g + 2)) // 2 + 1)
            nc.tensor.matmul(
                ots[g][:, off:512],
                lhsT=v_sb[:, j * 128 : (j + 1) * 128],
                rhs=pts[g][:, j, off:512],
                start=(j == 0),
                stop=(j == nblk - 1),
            )

        def epilogue(g):
            otsb = osb_pool.tile([128, 512], F32, tag="otsb", name="otsb")
            nc.vector.tensor_copy(otsb, ots[g])
            nc.sync.dma_start(out=y[:, g * 512 : (g + 1) * 512], in_=otsb)

        # ---- emission schedule: fast-start order, PV drains between iters ----
        order = [3] + list(range(NTB - 1, 3, -1)) + [2, 1, 0]
        pv_queue = []
        epi_pending = []
        state = {"done": set(), "ready": []}

        nv = {"v": 0}

        def drain(n):
            for _ in range(n):
                if not pv_queue or pv_queue[0][1] >= 4 * nv["v"]:
                    break
                g, j = pv_queue.pop(0)
                pv_job(g, j)
                if j == 8 * g + 7:
                    epi_pending.append(g)

        q_proj(0)  # qT cols for i=3 live in chunk 0
        for g in range(1, 4):
            q_proj(g)

        for idx, i in enumerate(order):
            for g in state["ready"]:
                pv_queue.extend((g, j) for j in range(8 * g + 8))
            state["ready"] = []
            if nv["v"] < 8:
                v_proj(nv["v"])
                nv["v"] += 1
            if epi_pending:
                epilogue(epi_pending.pop(0))
            qk_exp_xbar(i)
            state["done"].add(i)
            g = i // 4
            if all(4 * g + m in state["done"] for m in range(4)):
                state["ready"].append(g)
            drain(13)
        for g in state["ready"]:
            pv_queue.extend((g, j) for j in range(8 * g + 8))
        nv["v"] = 8
        while pv_queue:
            drain(10)
            if epi_pending:
                epilogue(epi_pending.pop(0))
        while epi_pending:
            epilogue(epi_pending.pop(0))
        # ship rowsum partials
        nc.sync.dma_start(out=rsum, in_=racc_sb.rearrange("p i g -> p (i g)"))

    nc.compile()
    return nc


def make_mask2(h: int) -> np.ndarray:
    m = np.zeros((128, 256), np.float32)
    ii = np.arange(128)[:, None]
    jj = np.arange(128)[None, :]
    tri = np.where(jj <= ii, 0.0, NEG).astype(np.float32)
    if h == 0:
        m[:, 0:128] = tri
        m[:, 128:256] = NEG
    else:
        m[:, 0:128] = 0.0
        m[:, 128:256] = tri
    return m.astype(ml_dtypes.bfloat16)


def own_rows(h: int) -> np.ndarray:
    blocks = 2 * np.arange(NTB) + h
    return (blocks[:, None] * 128 + np.arange(128)[None, :]).reshape(-1)


def prep_inputs(X, Wq, Wk, Wv):
    bf = ml_dtypes.bfloat16
    in_maps = []
    for c in range(N_CORES):
        b, h = c // 2, c % 2
        rows = own_rows(h)
        xb = np.asarray(X[b], np.float32)
        in_maps.append(
            {
                "xt": np.ascontiguousarray(xb.T).astype(bf),
                "xq": np.ascontiguousarray(xb[rows].T).astype(bf),
                "cpack": np.concatenate(
                    [
                        make_mask2(h),
                        np.eye(128, dtype=np.float32).astype(bf),
                        (np.asarray(Wq, np.float32).T @ np.asarray(Wk, np.float32)).astype(bf),
                        np.ascontiguousarray(np.asarray(Wv, np.float32).T).astype(bf),
                    ],
                    axis=1,
                ),
            }
        )
    return in_maps


def kernel(X, Wq, Wk, Wv, _trace=False, _tmpdir=None):
    if "nc" not in _cached:
        _cached["nc"] = build()
    nc = _cached["nc"]
    in_maps = prep_inputs(X, Wq, Wk, Wv)
    res = run_bass_kernel_spmd(
        nc,
        in_maps,
        list(range(N_CORES)),
        trace=_trace,
        **({"tmpdir": _tmpdir} if _tmpdir else {}),
    )
    out = np.empty((B, T, C), np.float32)
    for c in range(N_CORES):
        b, h = c // 2, c % 2
        ot = res.results[c]["y"]  # [128, 2048] = O^T unnormalized
        rp = res.results[c]["rsum"].reshape(128, NTB, 4)  # partial rowsums
        r = np.zeros((128, NTB), np.float32)
        for i in range(NTB):
            r[:, i] = rp[:, i, : ngr_of(i)].sum(axis=1)
        o = np.ascontiguousarray(ot.T).reshape(NTB, 128, C) / r.T[:, :, None]
        out[b, own_rows(h)] = o.reshape(NTB * 128, C)
    _cached["last_res"] = res
    return out


# revision 43
# speedup vs baseline: 1.0547x; 1.0355x over previous
"""Causal single-head attention (B=4, T=4096, C=128) on 8 Trainium2 cores.

Sharding: core c -> (batch b = c//2, parity h = c%2). Each core owns the
query row-blocks {128*(2i+h) : i=0..15} of its batch — an interleaved split
that balances causal work exactly and keeps the SPMD program identical
across cores (all per-core differences are input data, incl. the causal
mask for the last two key blocks of each row-block).

Per query block tb = 2i+h the core processes keys s < 128*(2i+2) (for h=0
the final key block is fully masked — wasted but uniform). Pipeline per
block i: QK^T matmuls (bf16, N=512 chunks into PSUM) -> causal mask added
by a tiny identity @ mask matmul accumulating into the same PSUM region ->
exp on ACT (PSUM -> SBUF bf16) with accumulated row-sum partials -> DMA-
xbar transpose of P into a 4-block group buffer. Per group of 4 blocks:
fat P^T @ V matmuls (V stationary, N up to 512) accumulate O^T[c, t] in
PSUM, which is copied out and shipped UNNORMALIZED and UNTRANSPOSED; the
host divides by the row sums and transposes. PV work for a group is
drained between later iterations so the in-order engine streams overlap.
"""

import sys

sys.path.insert(0, "/opt/trn_rl_repo")

import numpy as np
import ml_dtypes
from contextlib import ExitStack

import concourse.bass as bass
import concourse.mybir as mybir
import concourse.tile as tile
from concourse import bacc
from concourse.bass_utils import run_bass_kernel_spmd

B, T, C = 4, 4096, 128
N_CORES = 8
NTB = 16  # own 128-row query blocks per core
BF16 = mybir.dt.bfloat16
F32 = mybir.dt.float32
SCALE = float(1.0 / np.sqrt(C))
NEG = -1.0e30
ST_COLS = 1024  # PSUM score-tile width (2 banks)

_cached = {}


def s_len_of(i: int) -> int:
    return 128 * (2 * i + 2)


def ngr_of(i: int) -> int:
    return (s_len_of(i) + ST_COLS - 1) // ST_COLS


def build():
    nc = bacc.Bacc("TRN2", target_bir_lowering=False, debug=False)

    xt = nc.dram_tensor("xt", [C, T], BF16, kind="ExternalInput").ap()
    xq = nc.dram_tensor("xq", [C, NTB * 128], BF16, kind="ExternalInput").ap()
    cpack = nc.dram_tensor("cpack", [128, 640], BF16, kind="ExternalInput").ap()
    # outputs: O^T per group (host transposes + normalizes) and rowsum partials
    y = nc.dram_tensor("y", [C, NTB * 128], F32, kind="ExternalOutput").ap()
    rsum = nc.dram_tensor("rsum", [128, NTB * 4], F32, kind="ExternalOutput").ap()

    with ExitStack() as ctx:
        tc = ctx.enter_context(tile.TileContext(nc))
        consts = ctx.enter_context(tc.tile_pool(name="consts", bufs=1))
        pnat_pool = ctx.enter_context(tc.tile_pool(name="pnat", bufs=5))
        pt_pool = ctx.enter_context(tc.tile_pool(name="pt", bufs=1))
        osb_pool = ctx.enter_context(tc.tile_pool(name="osb", bufs=2))
        st_pool = ctx.enter_context(tc.tile_pool(name="st", bufs=3, space="PSUM"))
        ot_pool = ctx.enter_context(tc.tile_pool(name="ot", bufs=2, space="PSUM"))

        # ---- persistent SBUF tensors ----
        xt_sb = consts.tile([C, T], BF16, tag="xt_sb")
        xq_sb = consts.tile([C, NTB * 128], BF16, tag="xq_sb")
        cp_sb = consts.tile([128, 640], BF16, tag="cp_sb")
        m2_sb = cp_sb[:, 0:256]
        id_sb = cp_sb[:, 256:384]
        mqk_sb = cp_sb[:, 384:512]
        wv_sb = cp_sb[:, 512:640]
        qT_sb = consts.tile([C, NTB * 128], BF16, tag="qT_sb")  # [c, t_local]
        v_sb = consts.tile([C, T], BF16, tag="v_sb")  # [s%128, s_blk*128 + c]
        racc_sb = consts.tile([128, NTB, 4], F32, tag="racc_sb")  # rowsum partials

        nc.sync.dma_start(out=cp_sb, in_=cpack)
        nc.sync.dma_start(out=xq_sb[:, :512], in_=xq[:, :512])
        nc.sync.dma_start(out=xq_sb[:, 512:], in_=xq[:, 512:])
        for g in range(2):
            nc.sync.dma_start(
                out=xt_sb[:, g * 2048 : (g + 1) * 2048],
                in_=xt[:, g * 2048 : (g + 1) * 2048],
            )

        # ---- projection emitters (interleaved into the main loop) ----
        # q'^T[c, t] = sum_i M[i, c] * XQ[i, t]   (M = Wq^T Wk folded on host;
        # the key side is then raw X^T, so there is no K projection at all)
        def q_proj(g):
            st = st_pool.tile([128, ST_COLS], F32, tag="st", name="st")
            nc.tensor.matmul(
                st[:, :512],
                lhsT=mqk_sb,
                rhs=xq_sb[:, g * 512 : (g + 1) * 512],
                start=True,
                stop=True,
            )
            nc.vector.tensor_copy(qT_sb[:, g * 512 : (g + 1) * 512], st[:, :512])

        # V natural: V[s0+p, c] = sum_cin XT[cin, s0+p] * WvT[cin, c]
        def v_proj(g):
            st = st_pool.tile([128, ST_COLS], F32, tag="st", name="st")
            for m in range(4):
                j = 4 * g + m
                nc.tensor.matmul(
                    st[:, m * 128 : (m + 1) * 128],
                    lhsT=xt_sb[:, j * 128 : (j + 1) * 128],
                    rhs=wv_sb,
                    start=True,
                    stop=True,
                )
            nc.vector.tensor_copy(v_sb[:, g * 512 : (g + 1) * 512], st[:, :512])

        # ---- main loop pieces ----
        pts = {}  # group -> pt tile

        def qk_exp_xbar(i):
            s_len = s_len_of(i)
            nblk = s_len // 128
            ngr = ngr_of(i)
            g, il = i // 4, i % 4

            p_nat = pnat_pool.tile([128, T], BF16, tag="p_nat")

            for gg in range(ngr):
                g0 = gg * ST_COLS
                glen = min(ST_COLS, s_len - g0)
                st = st_pool.tile([128, ST_COLS], F32, tag="st")
                off = 0
                while off < glen:
                    w = min(512, glen - off)
                    last = gg == ngr - 1 and off + w == glen
                    nc.tensor.matmul(
                        st[:, off : off + w],
                        lhsT=qT_sb[:, i * 128 : (i + 1) * 128],
                        rhs=xt_sb[:, g0 + off : g0 + off + w],
                        start=True,
                        stop=not last,
                        skip_group_check=True,
                    )
                    off += w
                if gg == ngr - 1:
                    # causal mask: st[:, glen-256:glen] += I^T @ m2
                    nc.tensor.matmul(
                        st[:, glen - 256 : glen],
                        lhsT=id_sb,
                        rhs=m2_sb,
                        start=False,
                        stop=True,
                        skip_group_check=True,
                    )
                nc.scalar.activation(
                    out=p_nat[:, g0 : g0 + glen],
                    in_=st[:, :glen],
                    func=mybir.ActivationFunctionType.Exp,
                    scale=SCALE,
                    accum_out=racc_sb[:, i, gg : gg + 1],
                )

            if g not in pts:
                pts[g] = pt_pool.tile(
                    [128, 8 * g + 8, 512], BF16, tag=f"pt{g}", name="pt"
                )
            # transpose P [t128, s_len] -> [s%128, s_blk, t128] into group buf
            nc.sync.dma_start_transpose(
                out=pts[g][:, :nblk, il * 128 : (il + 1) * 128],
                in_=p_nat[:, :s_len],
            )

        ots = {}

        def pv_job(g, j):
            """One P^T@V matmul for group g, key-block j."""
            nblk = 8 * g + 8
            if j == 0:
                ots[g] = ot_pool.tile([128, 512], F32, tag="ot", name="ot")
            off = 0 if j < 8 * g + 2 else 128 * ((j - (8 * g + 2)) // 2 + 1)
            nc.tensor.matmul(
                ots[g][:, off:512],
                lhsT=v_sb[:, j * 128 : (j + 1) * 128],
                rhs=pts[g][:, j, off:512],
                start=(j == 0),
                stop=(j == nblk - 1),
            )

        def epilogue(g):
            otsb = osb_pool.tile([128, 512], F32, tag="otsb", name="otsb")
            nc.vector.tensor_copy(otsb, ots[g])
            nc.sync.dma_start(out=y[:, g * 512 : (g + 1) * 512], in_=otsb)

        # ---- emission schedule: fast-start order, PV drains between iters ----
        order = [3, 15, 14, 13, 12, 7, 6, 5, 4, 11, 10, 9, 8, 2, 1, 0]
        pv_queue = []
        epi_pending = []
        state = {"done": set(), "ready": []}

        nv = {"v": 0}

        def drain(n):
            for _ in range(n):
                if not pv_queue or pv_queue[0][1] >= 4 * nv["v"]:
                    break
                g, j = pv_queue.pop(0)
                pv_job(g, j)
                if j == 8 * g + 7:
                    epi_pending.append(g)

        q_proj(0)  # qT cols for i=3 live in chunk 0
        for g in range(1, 4):
            q_proj(g)

        for idx, i in enumerate(order):
            for g in state["ready"]:
                pv_queue.extend((g, j) for j in range(8 * g + 8))
            state["ready"] = []
            if nv["v"] < 8:
                v_proj(nv["v"])
                nv["v"] += 1
            if epi_pending:
                epilogue(epi_pending.pop(0))
            qk_exp_xbar(i)
            state["done"].add(i)
            g = i // 4
            if all(4 * g + m in state["done"] for m in range(4)):
                state["ready"].append(g)
            drain(16)
        for g in state["ready"]:
            pv_queue.extend((g, j) for j in range(8 * g + 8))
        nv["v"] = 8
        while pv_queue:
            drain(10)
            if epi_pending:
                epilogue(epi_pending.pop(0))
        while epi_pending:
            epilogue(epi_pending.pop(0))
        # ship rowsum partials
        nc.sync.dma_start(out=rsum, in_=racc_sb.rearrange("p i g -> p (i g)"))

    nc.compile()
    return nc


def make_mask2(h: int) -> np.ndarray:
    m = np.zeros((128, 256), np.float32)
    ii = np.arange(128)[:, None]
    jj = np.arange(128)[None, :]
    tri = np.where(jj <= ii, 0.0, NEG).astype(np.float32)
    if h == 0:
        m[:, 0:128] = tri
        m[:, 128:256] = NEG
    else:
        m[:, 0:128] = 0.0
        m[:, 128:256] = tri
    return m.astype(ml_dtypes.bfloat16)


def own_rows(h: int) -> np.ndarray:
    blocks = 2 * np.arange(NTB) + h
    return (blocks[:, None] * 128 + np.arange(128)[None, :]).reshape(-1)


def prep_inputs(X, Wq, Wk, Wv):
    bf = ml_dtypes.bfloat16
    in_maps = []
    for c in range(N_CORES):
        b, h = c // 2, c % 2
        rows = own_rows(h)
        xb = np.asarray(X[b], np.float32)
        in_maps.append(
            {
                "xt": np.ascontiguousarray(xb.T).astype(bf),
                "xq": np.ascontiguousarray(xb[rows].T).astype(bf),
                "cpack": np.concatenate(
                    [
                        make_mask2(h),
                        np.eye(128, dtype=np.float32).astype(bf),
                        (np.asarray(Wq, np.float32).T @ np.asarray(Wk, np.float32)).astype(bf),
                        np.ascontiguousarray(np.asarray(Wv, np.float32).T).astype(bf),
                    ],
                    axis=1,
                ),
            }
        )
    return in_maps


def kernel(X, Wq, Wk, Wv, _trace=False, _tmpdir=None):
    if "nc" not in _cached:
        _cached["nc"] = build()
    nc = _cached["nc"]
    in_maps = prep_inputs(X, Wq, Wk, Wv)
    res = run_bass_kernel_spmd(
        nc,
        in_maps,
        list(range(N_CORES)),
        trace=_trace,
        **({"tmpdir": _tmpdir} if _tmpdir else {}),
    )
    out = np.empty((B, T, C), np.float32)
    for c in range(N_CORES):
        b, h = c // 2, c % 2
        ot = res.results[c]["y"]  # [128, 2048] = O^T unnormalized
        rp = res.results[c]["rsum"].reshape(128, NTB, 4)  # partial rowsums
        r = np.zeros((128, NTB), np.float32)
        for i in range(NTB):
            r[:, i] = rp[:, i, : ngr_of(i)].sum(axis=1)
        o = np.ascontiguousarray(ot.T).reshape(NTB, 128, C) / r.T[:, :, None]
        out[b, own_rows(h)] = o.reshape(NTB * 128, C)
    _cached["last_res"] = res
    return out
